# revision 5
# baseline (speedup 1.0000x reference)
"""Trainium2 Bass kernel for nn_BDFM_Multi (B=8,C=256,H=W=128,N=4).

Data-parallel over batch: one batch element per NeuronCore (8 cores).

Per-core computation (feature f [C,HW], m [N,H,W], HW=16384):
  z    = (m > 0.3)                                  binary
  er   = 13-tap separable min-filter(z), dl = 13-tap separable max-filter(z)
         (composition of 4 iters of 4x4 cv2-style erode/dilate)
         -> computed as banded 0/1 matmuls + thresholds (exact on binary data)
  fbu  = per-class channels (er, 1-dl, dl-er)       [12, HW]
  mid  = fbu @ f^T                                  [12, C]
  cf   = Wf' @ f                                    [C, HW]   (BN-folded 1x1 conv)
  mid1 = mid @ cf                                   [12, HW]
  out  = Wo1' @ f + A' @ mid1 + u                   [C, HW]
         A' = Wo2' @ mid^T,  u = A' @ (mid @ beta_f) + beta_o
  (algebraic refactor of out = BN(Wo @ [f; mid^T @ (mid @ BN(Wf@f))]))

All big matmuls run in float32r (~1.5e-4 rel err, full PE rate).
"""
import numpy as np
from contextlib import ExitStack

import concourse.bass as bass
import concourse.mybir as mybir
import concourse.tile as tile
from concourse import bacc
from concourse import bass_utils
from concourse.masks import make_identity

F32 = mybir.dt.float32
F32R = mybir.dt.float32r
ALU = mybir.AluOpType
ACTF = mybir.ActivationFunctionType

B, C, H, W, N = 8, 256, 128, 128, 4
HW = H * W
EPS = 1e-5
P = 128
PT = 512              # p-tile width for pass 2
NPT = HW // PT        # 32 p-tiles

_NC_CACHE = {}


def _band_consts():
    idx = np.arange(P)
    # erosion: output i covers input [i-8, i+4]; dilation: [i-4, i+8]
    band_er = ((idx[:, None] >= idx[None, :] - 8) &
               (idx[:, None] <= idx[None, :] + 4)).astype(np.float32)
    band_dl = ((idx[:, None] >= idx[None, :] - 4) &
               (idx[:, None] <= idx[None, :] + 8)).astype(np.float32)
    cnt_er = band_er.sum(axis=0, dtype=np.float32).reshape(P, 1)
    return band_er, band_dl, cnt_er


def build():
    if "nc" in _NC_CACHE:
        return _NC_CACHE["nc"]
    nc = bacc.Bacc(trn_type="TRN2", target_bir_lowering=False, debug=False)

    feature = nc.dram_tensor("feature", [C, HW], F32, kind="ExternalInput")
    m_in = nc.dram_tensor("m", [N, H, W], F32, kind="ExternalInput")
    wf_t = nc.dram_tensor("wf_t", [P, 512], F32, kind="ExternalInput")    # blocks ci*2+o
    wo1_t = nc.dram_tensor("wo1_t", [P, 512], F32, kind="ExternalInput")  # blocks ci*2+o
    wo2t = nc.dram_tensor("wo2t", [P, 512], F32, kind="ExternalInput")    # blocks ci -> 256 cols
    beta_f = nc.dram_tensor("beta_f", [P, 2], F32, kind="ExternalInput")  # col ci
    beta_o = nc.dram_tensor("beta_o", [P, 2], F32, kind="ExternalInput")  # col o
    band_er = nc.dram_tensor("band_er", [P, P], F32, kind="ExternalInput")
    band_dl = nc.dram_tensor("band_dl", [P, P], F32, kind="ExternalInput")
    cnt_er = nc.dram_tensor("cnt_er", [P, 1], F32, kind="ExternalInput")
    out = nc.dram_tensor("out", [C, HW], F32, kind="ExternalOutput")

    with tile.TileContext(nc) as tc, ExitStack() as ctx:
        persist = ctx.enter_context(tc.tile_pool(name="persist", bufs=1))

        # ---------------- loads ----------------
        feat = persist.tile([P, 2 * HW], F32R)   # c-blk0 | c-blk1, cast fp32->fp32r
        for blk in range(2):
            for q in range(4):
                nc.gpsimd.dma_start(
                    out=feat[:, blk * HW + q * 4096: blk * HW + (q + 1) * 4096],
                    in_=feature[blk * P:(blk + 1) * P, q * 4096:(q + 1) * 4096])

        m_sb = persist.tile([P, N * P], F32)
        for n in range(N):
            nc.sync.dma_start(out=m_sb[:, n * P:(n + 1) * P], in_=m_in[n])

        wf_sb = persist.tile([P, 512], F32R)
        nc.gpsimd.dma_start(out=wf_sb[:], in_=wf_t[:])
        wo1_sb = persist.tile([P, 512], F32R)
        nc.gpsimd.dma_start(out=wo1_sb[:], in_=wo1_t[:])
        wo2_sb = persist.tile([P, 512], F32R)
        nc.gpsimd.dma_start(out=wo2_sb[:], in_=wo2t[:])
        betaf_sb = persist.tile([P, 2], F32)
        nc.sync.dma_start(out=betaf_sb[:], in_=beta_f[:])
        betao_sb = persist.tile([P, 2], F32)
        nc.sync.dma_start(out=betao_sb[:], in_=beta_o[:])
        bander_sb = persist.tile([P, P], F32R)
        nc.gpsimd.dma_start(out=bander_sb[:], in_=band_er[:])
        banddl_sb = persist.tile([P, P], F32R)
        nc.gpsimd.dma_start(out=banddl_sb[:], in_=band_dl[:])
        cnt_sb = persist.tile([P, 1], F32)
        nc.sync.dma_start(out=cnt_sb[:], in_=cnt_er[:])

        ident = persist.tile([P, P], F32)
        make_identity(nc, ident)
        ident_r = persist.tile([P, P], F32R)
        nc.vector.tensor_copy(ident_r[:], ident[:])

        # persistent results of phase 1
        fbuT = persist.tile([P, P * 12], F32R)     # [w, h*12 + k]
        mid_t_sb = persist.tile([P, 24], F32R)     # mid^T chunks [c-chunk, 12] x2
        a_t_sb = persist.tile([12, 256], F32R)     # A'^T
        u_sb = persist.tile([P, 2], F32)           # bias per o-blk

        # ---------------- morphology ----------------
        with tc.tile_pool(name="morph", bufs=1) as mo, \
             tc.tile_pool(name="morph_ps", bufs=2, space="PSUM") as mops:
            z_sb = mo.tile([P, N * P], F32R)
            nc.vector.tensor_scalar(z_sb[:], m_sb[:], 0.3, None, op0=ALU.is_gt)

            ps_rows_er = mops.tile([P, N * P], F32, tag="mps")
            nc.tensor.matmul(ps_rows_er[:], bander_sb[:], z_sb[:],
                             start=True, stop=True)
            ps_rows_dl = mops.tile([P, N * P], F32, tag="mps")
            nc.tensor.matmul(ps_rows_dl[:], banddl_sb[:], z_sb[:],
                             start=True, stop=True)

            rows_er = mo.tile([P, N * P], F32R)
            nc.vector.tensor_scalar(rows_er[:], ps_rows_er[:], cnt_sb[:, 0:1], None,
                                    op0=ALU.is_equal)
            rows_dl = mo.tile([P, N * P], F32R)
            nc.vector.tensor_scalar(rows_dl[:], ps_rows_dl[:], 0.5, None,
                                    op0=ALU.is_gt)

            # transpose each class tile -> [w, h]
            rows_erT = mo.tile([P, N * P], F32R)
            rows_dlT = mo.tile([P, N * P], F32R)
            for n in range(N):
                ps_tr = mops.tile([P, 2 * P], F32R, tag="mps")
                nc.tensor.matmul(ps_tr[:, 0:P], rows_er[:, n * P:(n + 1) * P],
                                 ident_r[:], is_transpose=True)
                nc.tensor.matmul(ps_tr[:, P:2 * P], rows_dl[:, n * P:(n + 1) * P],
                                 ident_r[:], is_transpose=True)
                nc.vector.tensor_copy(rows_erT[:, n * P:(n + 1) * P], ps_tr[:, 0:P])
                nc.vector.tensor_copy(rows_dlT[:, n * P:(n + 1) * P], ps_tr[:, P:2 * P])

            ps_cols_er = mops.tile([P, N * P], F32, tag="mps")
            nc.tensor.matmul(ps_cols_er[:], bander_sb[:], rows_erT[:],
                             start=True, stop=True)
            ps_cols_dl = mops.tile([P, N * P], F32, tag="mps")
            nc.tensor.matmul(ps_cols_dl[:], banddl_sb[:], rows_dlT[:],
                             start=True, stop=True)

            er_t = mo.tile([P, N * P], F32)   # er^T per class [w, h]
            dl_t = mo.tile([P, N * P], F32)
            nc.vector.tensor_scalar(er_t[:], ps_cols_er[:], cnt_sb[:, 0:1], None,
                                    op0=ALU.is_equal)
            nc.vector.tensor_scalar(dl_t[:], ps_cols_dl[:], 0.5, None, op0=ALU.is_gt)

            # write channels into fbuT at [w, h*12 + k], k = 3n+j
            for n in range(N):
                src_er = er_t[:, n * P:(n + 1) * P]
                src_dl = dl_t[:, n * P:(n + 1) * P]
                fbuT_v = fbuT.rearrange("w (h k) -> w h k", k=12)
                dst_er = fbuT_v[:, :, 3 * n]
                dst_bg = fbuT_v[:, :, 3 * n + 1]
                dst_md = fbuT_v[:, :, 3 * n + 2]
                nc.vector.tensor_copy(dst_er, src_er)
                nc.vector.tensor_scalar(dst_bg, src_dl, 0.0, None, op0=ALU.is_equal)
                nc.vector.tensor_tensor(dst_md, src_dl, src_er, op=ALU.subtract)

        # ---------------- pass 1: feature transpose + mid ----------------
        with tc.tile_pool(name="mid_ps", bufs=1, space="PSUM") as midps, \
             tc.tile_pool(name="p1_ps", bufs=3, space="PSUM") as p1ps, \
             tc.tile_pool(name="p1s_ps", bufs=1, space="PSUM") as p1s, \
             tc.tile_pool(name="p1_sb", bufs=3) as p1sb:
            mid_ps = midps.tile([12, 256], F32)
            for h in range(P):
                tr = p1ps.tile([P, 256], F32R, tag="tr")
                nc.tensor.matmul(tr[:, 0:P], feat[:, h * P:(h + 1) * P],
                                 ident_r[:], is_transpose=True)
                nc.tensor.matmul(tr[:, P:256], feat[:, HW + h * P:HW + (h + 1) * P],
                                 ident_r[:], is_transpose=True)
                ft = p1sb.tile([P, 256], F32R, tag="ft")
                nc.scalar.copy(ft[:], tr[:])
                nc.tensor.matmul(mid_ps[:], fbuT[:, h * 12:h * 12 + 12], ft[:],
                                 start=(h == 0), stop=(h == P - 1),
                                 skip_group_check=True)

            mid_sb = persist.tile([12, 256], F32)
            nc.vector.tensor_copy(mid_sb[:], mid_ps[:])

            # mid^T via PE transpose of [12,128] chunks
            ps_mt = p1s.tile([P, 24], F32, tag="mt")
            for ci in range(2):
                nc.tensor.matmul(ps_mt[:, ci * 12:(ci + 1) * 12],
                                 mid_sb[:, ci * P:(ci + 1) * P],
                                 ident[0:12, 0:12], is_transpose=True)
            nc.vector.tensor_copy(mid_t_sb[:], ps_mt[:])
            mid_tf = persist.tile([P, 24], F32)
            nc.vector.tensor_copy(mid_tf[:], ps_mt[:])

            # A'^T = mid @ Wo2'^T   [12, 256]
            ps_at = p1s.tile([12, 256], F32, tag="at")
            nc.tensor.matmul(ps_at[:], mid_t_sb[:, 0:12], wo2_sb[:, 0:256],
                             start=True, stop=False)
            nc.tensor.matmul(ps_at[:], mid_t_sb[:, 12:24], wo2_sb[:, 256:512],
                             start=False, stop=True)
            nc.vector.tensor_copy(a_t_sb[:], ps_at[:])
            a_tf = persist.tile([12, 256], F32)
            nc.vector.tensor_copy(a_tf[:], ps_at[:])

            # s = mid @ beta_f   [12, 1]
            ps_s = p1s.tile([12, 1], F32, tag="s")
            nc.tensor.matmul(ps_s[:], mid_tf[:, 0:12], betaf_sb[:, 0:1],
                             start=True, stop=False)
            nc.tensor.matmul(ps_s[:], mid_tf[:, 12:24], betaf_sb[:, 1:2],
                             start=False, stop=True)
            s_sb = persist.tile([12, 1], F32)
            nc.vector.tensor_copy(s_sb[:], ps_s[:])

            # u = A' @ s + beta_o   per o-blk
            for o in range(2):
                ps_u = p1s.tile([P, 1], F32, tag="u")
                nc.tensor.matmul(ps_u[:], a_tf[:, o * P:(o + 1) * P], s_sb[:],
                                 start=True, stop=True)
                nc.scalar.activation(u_sb[:, o:o + 1], ps_u[:], ACTF.Identity,
                                     bias=betao_sb[:, o:o + 1])

        # ---------------- pass 2: cf, mid1, out ----------------
        with tc.tile_pool(name="cf_ps", bufs=1, space="PSUM") as cfps, \
             tc.tile_pool(name="m1_ps", bufs=2, space="PSUM") as m1ps, \
             tc.tile_pool(name="out_ps", bufs=4, space="PSUM") as outps, \
             tc.tile_pool(name="p2_sb", bufs=2) as p2sb:
            for t in range(NPT):
                c0 = t * PT
                cf_ps = cfps.tile([P, 2 * PT], F32, tag="cf")
                for o in range(2):
                    nc.tensor.matmul(cf_ps[:, o * PT:(o + 1) * PT],
                                     wf_sb[:, (0 * 2 + o) * P:(0 * 2 + o + 1) * P],
                                     feat[:, c0:c0 + PT],
                                     start=True, stop=False, skip_group_check=True)
                    nc.tensor.matmul(cf_ps[:, o * PT:(o + 1) * PT],
                                     wf_sb[:, (1 * 2 + o) * P:(1 * 2 + o + 1) * P],
                                     feat[:, HW + c0:HW + c0 + PT],
                                     start=False, stop=True, skip_group_check=True)
                cf = p2sb.tile([P, 2 * PT], F32R, tag="cf_sb")
                nc.vector.tensor_copy(cf[:], cf_ps[:])

                m1_ps = m1ps.tile([12, PT], F32, tag="m1")
                nc.tensor.matmul(m1_ps[:], mid_t_sb[:, 0:12], cf[:, 0:PT],
                                 start=True, stop=False)
                nc.tensor.matmul(m1_ps[:], mid_t_sb[:, 12:24], cf[:, PT:2 * PT],
                                 start=False, stop=True)
                m1 = p2sb.tile([12, PT], F32R, tag="m1_sb")
                nc.vector.tensor_copy(m1[:], m1_ps[:])

                ot = p2sb.tile([P, 2 * PT], F32, tag="ot")
                for o in range(2):
                    o_ps = outps.tile([P, PT], F32, tag="ops")
                    nc.tensor.matmul(o_ps[:],
                                     wo1_sb[:, (0 * 2 + o) * P:(0 * 2 + o + 1) * P],
                                     feat[:, c0:c0 + PT],
                                     start=True, stop=False, skip_group_check=True)
                    nc.tensor.matmul(o_ps[:],
                                     wo1_sb[:, (1 * 2 + o) * P:(1 * 2 + o + 1) * P],
                                     feat[:, HW + c0:HW + c0 + PT],
                                     start=False, stop=False, skip_group_check=True)
                    nc.tensor.matmul(o_ps[:],
                                     a_t_sb[:, o * P:(o + 1) * P], m1[:],
                                     start=False, stop=True, skip_group_check=True)
                    nc.scalar.activation(ot[:, o * PT:(o + 1) * PT], o_ps[:],
                                         ACTF.Identity, bias=u_sb[:, o:o + 1])
                nc.sync.dma_start(out=out[0:P, c0:c0 + PT], in_=ot[:, 0:PT])
                nc.sync.dma_start(out=out[P:C, c0:c0 + PT], in_=ot[:, PT:2 * PT])

    nc.compile()
    _NC_CACHE["nc"] = nc
    return nc


def kernel(feature, m, W_f, g_f, b_f, mu_f, v_f, W_o, g_o, b_o, mu_o, v_o):
    feature = np.asarray(feature, dtype=np.float32)
    m = np.asarray(m, dtype=np.float32)
    W_f = np.asarray(W_f, dtype=np.float32)
    W_o = np.asarray(W_o, dtype=np.float32)
    g_f, b_f, mu_f, v_f = (np.asarray(x, dtype=np.float32) for x in (g_f, b_f, mu_f, v_f))
    g_o, b_o, mu_o, v_o = (np.asarray(x, dtype=np.float32) for x in (g_o, b_o, mu_o, v_o))

    inv_f = g_f / np.sqrt(v_f + EPS)
    beta_f_v = b_f - mu_f * inv_f
    inv_o = g_o / np.sqrt(v_o + EPS)
    beta_o_v = b_o - mu_o * inv_o
    Wf_p = (inv_f[:, None] * W_f).astype(np.float32)          # [C, C]
    Wo1_p = (inv_o[:, None] * W_o[:, :C]).astype(np.float32)  # [C, C]
    Wo2_p = (inv_o[:, None] * W_o[:, C:]).astype(np.float32)  # [C, C]

    # lhsT layouts: blocks ci*2+o of W'^T
    def blocks_t(Wp):
        a = np.empty((P, 512), np.float32)
        for ci in range(2):
            for o in range(2):
                a[:, (ci * 2 + o) * P:(ci * 2 + o + 1) * P] = \
                    Wp[o * P:(o + 1) * P, ci * P:(ci + 1) * P].T
        return a

    wf_t_np = blocks_t(Wf_p)
    wo1_t_np = blocks_t(Wo1_p)
    wo2t_np = np.concatenate([Wo2_p.T[0:P, :], Wo2_p.T[P:C, :]], axis=1)  # [128, 512]
    beta_f_np = beta_f_v.reshape(2, P).T.copy()   # col ci = beta_f[ci*128:(ci+1)*128]
    beta_o_np = beta_o_v.reshape(2, P).T.copy()
    band_er, band_dl, cnt_er = _band_consts()

    nc = build()
    common = {
        "wf_t": wf_t_np, "wo1_t": wo1_t_np, "wo2t": wo2t_np,
        "beta_f": beta_f_np, "beta_o": beta_o_np,
        "band_er": band_er, "band_dl": band_dl, "cnt_er": cnt_er,
    }
    in_maps = []
    for b in range(B):
        im = dict(common)
        im["feature"] = np.ascontiguousarray(feature[b].reshape(C, HW))
        im["m"] = np.ascontiguousarray(m[b])
        in_maps.append(im)

    res = bass_utils.run_bass_kernel_spmd(nc, in_maps, list(range(B)))
    out = np.empty((B, C, H, W), np.float32)
    for b in range(B):
        out[b] = res.results[b]["out"].reshape(C, H, W)
    return out


# revision 6
# speedup vs baseline: 1.0725x; 1.0725x over previous
"""Trainium2 Bass kernel for nn_BDFM_Multi (B=8,C=256,H=W=128,N=4).

Data-parallel over batch: one batch element per NeuronCore (8 cores).

Per-core computation (feature f [C,HW], m [N,H,W], HW=16384):
  z    = (m > 0.3)                                  binary
  er   = 13-tap separable min-filter(z), dl = 13-tap separable max-filter(z)
         (composition of 4 iters of 4x4 cv2-style erode/dilate)
         -> computed as banded 0/1 matmuls + thresholds (exact on binary data)
  fbu  = per-class channels (er, 1-dl, dl-er)       [12, HW]
  mid  = fbu @ f^T                                  [12, C]
  A'   = Wo2' @ mid^T                               [C, 12]
  G    = A' @ mid                                   [C, C]
  cf   = Wf' @ f                                    [C, HW]   (BN-folded 1x1 conv)
  u    = G @ beta_f + beta_o                        [C]
  out  = Wo1' @ f + G @ cf + u                      [C, HW]
  (algebraic refactor of out = BN(Wo @ [f; mid^T @ (mid @ BN(Wf@f))]))

All big matmuls run in float32r (~2e-4 rel err, full PE rate).
"""
import numpy as np
from contextlib import ExitStack

import concourse.bass as bass
import concourse.mybir as mybir
import concourse.tile as tile
from concourse import bacc
from concourse import bass_utils
from concourse.masks import make_identity

F32 = mybir.dt.float32
F32R = mybir.dt.float32r
ALU = mybir.AluOpType
ACTF = mybir.ActivationFunctionType

B, C, H, W, N = 8, 256, 128, 128, 4
HW = H * W
EPS = 1e-5
P = 128
PT = 512              # p-tile width for pass 2
NPT = HW // PT        # 32 p-tiles
G1 = 4                # h-chunks per transpose group in pass 1

_NC_CACHE = {}


def _band_consts():
    idx = np.arange(P)
    # erosion: output i covers input [i-8, i+4]; dilation: [i-4, i+8]
    band_er = ((idx[:, None] >= idx[None, :] - 8) &
               (idx[:, None] <= idx[None, :] + 4)).astype(np.float32)
    band_dl = ((idx[:, None] >= idx[None, :] - 4) &
               (idx[:, None] <= idx[None, :] + 8)).astype(np.float32)
    cnt_er = band_er.sum(axis=0, dtype=np.float32).reshape(P, 1)
    return band_er, band_dl, cnt_er


def build():
    if "nc" in _NC_CACHE:
        return _NC_CACHE["nc"]
    nc = bacc.Bacc(trn_type="TRN2", target_bir_lowering=False, debug=False)

    feature = nc.dram_tensor("feature", [C, HW], F32, kind="ExternalInput")
    m_in = nc.dram_tensor("m", [N, H, W], F32, kind="ExternalInput")
    wf_t = nc.dram_tensor("wf_t", [P, 512], F32, kind="ExternalInput")    # blocks ci*2+o
    wo1_t = nc.dram_tensor("wo1_t", [P, 512], F32, kind="ExternalInput")  # blocks ci*2+o
    wo2t = nc.dram_tensor("wo2t", [P, 512], F32, kind="ExternalInput")    # blocks ci -> 256 cols
    beta_f = nc.dram_tensor("beta_f", [P, 2], F32, kind="ExternalInput")  # col ci
    beta_o = nc.dram_tensor("beta_o", [P, 2], F32, kind="ExternalInput")  # col o
    band_er = nc.dram_tensor("band_er", [P, P], F32, kind="ExternalInput")
    band_dl = nc.dram_tensor("band_dl", [P, P], F32, kind="ExternalInput")
    cnt_er = nc.dram_tensor("cnt_er", [P, 1], F32, kind="ExternalInput")
    out = nc.dram_tensor("out", [C, HW], F32, kind="ExternalOutput")

    with tile.TileContext(nc) as tc, ExitStack() as ctx:
        persist = ctx.enter_context(tc.tile_pool(name="persist", bufs=1))

        # ---------------- loads ----------------
        feat = persist.tile([P, 2 * HW], F32R)   # c-blk0 | c-blk1, cast fp32->fp32r
        for blk in range(2):
            for q in range(4):
                nc.gpsimd.dma_start(
                    out=feat[:, blk * HW + q * 4096: blk * HW + (q + 1) * 4096],
                    in_=feature[blk * P:(blk + 1) * P, q * 4096:(q + 1) * 4096])

        m_sb = persist.tile([P, N * P], F32)
        for n in range(N):
            nc.sync.dma_start(out=m_sb[:, n * P:(n + 1) * P], in_=m_in[n])

        wf_sb = persist.tile([P, 512], F32R)
        nc.gpsimd.dma_start(out=wf_sb[:], in_=wf_t[:])
        wo1_sb = persist.tile([P, 512], F32R)
        nc.gpsimd.dma_start(out=wo1_sb[:], in_=wo1_t[:])
        wo2_sb = persist.tile([P, 512], F32R)
        nc.gpsimd.dma_start(out=wo2_sb[:], in_=wo2t[:])
        betaf_sb = persist.tile([P, 2], F32)
        nc.sync.dma_start(out=betaf_sb[:], in_=beta_f[:])
        betao_sb = persist.tile([P, 2], F32)
        nc.sync.dma_start(out=betao_sb[:], in_=beta_o[:])
        bander_sb = persist.tile([P, P], F32R)
        nc.gpsimd.dma_start(out=bander_sb[:], in_=band_er[:])
        banddl_sb = persist.tile([P, P], F32R)
        nc.gpsimd.dma_start(out=banddl_sb[:], in_=band_dl[:])
        cnt_sb = persist.tile([P, 1], F32)
        nc.sync.dma_start(out=cnt_sb[:], in_=cnt_er[:])

        ident = persist.tile([P, P], F32)
        make_identity(nc, ident)
        ident_r = persist.tile([P, P], F32R)
        nc.vector.tensor_copy(ident_r[:], ident[:])

        # persistent results of phase 1
        fbuT = persist.tile([P, P * 12], F32R)     # [w, h*12 + k]
        gt_sb = persist.tile([P, 512], F32R)       # G^T blocks [c, ci*256 + o*128]
        gt_f = persist.tile([P, 512], F32)         # fp32 copy for the tiny u matmul
        u_sb = persist.tile([P, 2], F32)           # bias per o-blk

        # ---------------- morphology ----------------
        with tc.tile_pool(name="morph", bufs=1) as mo, \
             tc.tile_pool(name="morph_ps", bufs=2, space="PSUM") as mops:
            z_sb = mo.tile([P, N * P], F32R)
            nc.vector.tensor_scalar(z_sb[:], m_sb[:], 0.3, None, op0=ALU.is_gt)

            ps_rows_er = mops.tile([P, N * P], F32, tag="mps")
            nc.tensor.matmul(ps_rows_er[:], bander_sb[:], z_sb[:],
                             start=True, stop=True)
            ps_rows_dl = mops.tile([P, N * P], F32, tag="mps")
            nc.tensor.matmul(ps_rows_dl[:], banddl_sb[:], z_sb[:],
                             start=True, stop=True)

            rows_er = mo.tile([P, N * P], F32R)
            nc.vector.tensor_scalar(rows_er[:], ps_rows_er[:], cnt_sb[:, 0:1], None,
                                    op0=ALU.is_equal)
            rows_dl = mo.tile([P, N * P], F32R)
            nc.vector.tensor_scalar(rows_dl[:], ps_rows_dl[:], 0.5, None,
                                    op0=ALU.is_gt)

            # transpose each class tile -> [w, h]
            rows_erT = mo.tile([P, N * P], F32R)
            rows_dlT = mo.tile([P, N * P], F32R)
            for n in range(N):
                ps_tr = mops.tile([P, 2 * P], F32R, tag="mps")
                nc.tensor.matmul(ps_tr[:, 0:P], rows_er[:, n * P:(n + 1) * P],
                                 ident_r[:], is_transpose=True)
                nc.tensor.matmul(ps_tr[:, P:2 * P], rows_dl[:, n * P:(n + 1) * P],
                                 ident_r[:], is_transpose=True)
                nc.vector.tensor_copy(rows_erT[:, n * P:(n + 1) * P], ps_tr[:, 0:P])
                nc.vector.tensor_copy(rows_dlT[:, n * P:(n + 1) * P], ps_tr[:, P:2 * P])

            ps_cols_er = mops.tile([P, N * P], F32, tag="mps")
            nc.tensor.matmul(ps_cols_er[:], bander_sb[:], rows_erT[:],
                             start=True, stop=True)
            ps_cols_dl = mops.tile([P, N * P], F32, tag="mps")
            nc.tensor.matmul(ps_cols_dl[:], banddl_sb[:], rows_dlT[:],
                             start=True, stop=True)

            er_t = mo.tile([P, N * P], F32)   # er^T per class [w, h]
            dl_t = mo.tile([P, N * P], F32)
            nc.vector.tensor_scalar(er_t[:], ps_cols_er[:], cnt_sb[:, 0:1], None,
                                    op0=ALU.is_equal)
            nc.vector.tensor_scalar(dl_t[:], ps_cols_dl[:], 0.5, None, op0=ALU.is_gt)

            # write channels into fbuT at [w, h*12 + k], k = 3n+j
            fbuT_v = fbuT.rearrange("w (h k) -> w h k", k=12)
            for n in range(N):
                src_er = er_t[:, n * P:(n + 1) * P]
                src_dl = dl_t[:, n * P:(n + 1) * P]
                nc.vector.tensor_copy(fbuT_v[:, :, 3 * n], src_er)
                nc.vector.tensor_scalar(fbuT_v[:, :, 3 * n + 1], src_dl, 0.0, None,
                                        op0=ALU.is_equal)
                nc.vector.tensor_tensor(fbuT_v[:, :, 3 * n + 2], src_dl, src_er,
                                        op=ALU.subtract)

        # ---------------- pass 1: feature transpose + mid ----------------
        mid_r = persist.tile([12, 256], F32R)
        with tc.tile_pool(name="mid_ps", bufs=1, space="PSUM") as midps, \
             tc.tile_pool(name="p1_ps", bufs=2, space="PSUM") as p1ps, \
             tc.tile_pool(name="p1_sb", bufs=3) as p1sb:
            mid_ps = midps.tile([12, 256], F32)
            for g in range(P // G1):
                tr = p1ps.tile([P, G1 * 256], F32R, tag="tr")
                for j in range(G1):
                    h = g * G1 + j
                    nc.tensor.matmul(tr[:, j * 256:j * 256 + P],
                                     feat[:, h * P:(h + 1) * P],
                                     ident_r[:], is_transpose=True)
                    nc.tensor.matmul(tr[:, j * 256 + P:(j + 1) * 256],
                                     feat[:, HW + h * P:HW + (h + 1) * P],
                                     ident_r[:], is_transpose=True)
                ft = p1sb.tile([P, G1 * 256], F32R, tag="ft")
                if g % 2 == 0:
                    nc.vector.tensor_copy(ft[:], tr[:])
                else:
                    nc.scalar.copy(ft[:], tr[:])
                for j in range(G1):
                    h = g * G1 + j
                    nc.tensor.matmul(mid_ps[:], fbuT[:, h * 12:h * 12 + 12],
                                     ft[:, j * 256:(j + 1) * 256],
                                     start=(h == 0), stop=(h == P - 1),
                                     skip_group_check=True)
            nc.vector.tensor_copy(mid_r[:], mid_ps[:])

        # ---------------- small stage: mid^T, A'^T, G^T, u ----------------
        with tc.tile_pool(name="sm_ps", bufs=1, space="PSUM") as smps, \
             tc.tile_pool(name="sm_sb", bufs=1) as smsb:
            # mid^T via PE transpose of [12,128] chunks (fp32r)
            ps_mt = smps.tile([P, 24], F32R, tag="mt")
            for ci in range(2):
                nc.tensor.matmul(ps_mt[:, ci * 12:(ci + 1) * 12],
                                 mid_r[:, ci * P:(ci + 1) * P],
                                 ident_r[0:12, 0:12], is_transpose=True)
            mid_t = smsb.tile([P, 24], F32R)
            nc.vector.tensor_copy(mid_t[:], ps_mt[:])

            # A'^T = mid @ Wo2'^T   [12, 256]
            ps_at = smps.tile([12, 256], F32, tag="at")
            nc.tensor.matmul(ps_at[:], mid_t[:, 0:12], wo2_sb[:, 0:256],
                             start=True, stop=False)
            nc.tensor.matmul(ps_at[:], mid_t[:, 12:24], wo2_sb[:, 256:512],
                             start=False, stop=True)
            a_t = smsb.tile([12, 256], F32R)
            nc.vector.tensor_copy(a_t[:], ps_at[:])

            # G^T[c, o] = sum_k mid[k, c] A'^T[k, o];  chunks ci on partitions
            ps_gt = smps.tile([P, 512], F32, tag="gt")
            for ci in range(2):
                nc.tensor.matmul(ps_gt[:, ci * 256:(ci + 1) * 256],
                                 mid_r[:, ci * P:(ci + 1) * P], a_t[:],
                                 start=True, stop=True)
            nc.vector.tensor_copy(gt_sb[:], ps_gt[:])
            nc.vector.tensor_copy(gt_f[:], ps_gt[:])

            # u = G @ beta_f + beta_o   per o-blk  (fp32 matmuls)
            for o in range(2):
                ps_u = smps.tile([P, 1], F32, tag="u")
                nc.tensor.matmul(ps_u[:], gt_f[:, o * P:(o + 1) * P],
                                 betaf_sb[:, 0:1], start=True, stop=False)
                nc.tensor.matmul(ps_u[:], gt_f[:, 256 + o * P:256 + (o + 1) * P],
                                 betaf_sb[:, 1:2], start=False, stop=True)
                nc.scalar.activation(u_sb[:, o:o + 1], ps_u[:], ACTF.Identity,
                                     bias=betao_sb[:, o:o + 1])

        # ---------------- pass 2: cf, out ----------------
        with tc.tile_pool(name="cf_ps", bufs=2, space="PSUM") as cfps, \
             tc.tile_pool(name="out_ps", bufs=2, space="PSUM") as outps, \
             tc.tile_pool(name="p2_sb", bufs=2) as p2sb:
            for t in range(NPT):
                c0 = t * PT
                cf_ps = cfps.tile([P, 2 * PT], F32, tag="cf")
                for o in range(2):
                    nc.tensor.matmul(cf_ps[:, o * PT:(o + 1) * PT],
                                     wf_sb[:, (0 * 2 + o) * P:(0 * 2 + o + 1) * P],
                                     feat[:, c0:c0 + PT],
                                     start=True, stop=False, skip_group_check=True)
                    nc.tensor.matmul(cf_ps[:, o * PT:(o + 1) * PT],
                                     wf_sb[:, (1 * 2 + o) * P:(1 * 2 + o + 1) * P],
                                     feat[:, HW + c0:HW + c0 + PT],
                                     start=False, stop=True, skip_group_check=True)
                cf = p2sb.tile([P, 2 * PT], F32R, tag="cf_sb")
                if t % 2 == 0:
                    nc.scalar.copy(cf[:], cf_ps[:])
                else:
                    nc.vector.tensor_copy(cf[:], cf_ps[:])

                ot = p2sb.tile([P, 2 * PT], F32, tag="ot")
                out_ps = outps.tile([P, 2 * PT], F32, tag="ops")
                for o in range(2):
                    ops = out_ps[:, o * PT:(o + 1) * PT]
                    nc.tensor.matmul(ops,
                                     wo1_sb[:, (0 * 2 + o) * P:(0 * 2 + o + 1) * P],
                                     feat[:, c0:c0 + PT],
                                     start=True, stop=False, skip_group_check=True)
                    nc.tensor.matmul(ops,
                                     wo1_sb[:, (1 * 2 + o) * P:(1 * 2 + o + 1) * P],
                                     feat[:, HW + c0:HW + c0 + PT],
                                     start=False, stop=False, skip_group_check=True)
                    nc.tensor.matmul(ops,
                                     gt_sb[:, o * P:(o + 1) * P],
                                     cf[:, 0:PT],
                                     start=False, stop=False, skip_group_check=True)
                    nc.tensor.matmul(ops,
                                     gt_sb[:, 256 + o * P:256 + (o + 1) * P],
                                     cf[:, PT:2 * PT],
                                     start=False, stop=True, skip_group_check=True)
                    if o == 0:
                        nc.scalar.activation(ot[:, o * PT:(o + 1) * PT], ops,
                                             ACTF.Identity, bias=u_sb[:, o:o + 1])
                    else:
                        nc.vector.tensor_scalar(ot[:, o * PT:(o + 1) * PT], ops,
                                                u_sb[:, o:o + 1], None, op0=ALU.add)
                nc.sync.dma_start(out=out[0:P, c0:c0 + PT], in_=ot[:, 0:PT])
                nc.sync.dma_start(out=out[P:C, c0:c0 + PT], in_=ot[:, PT:2 * PT])

    nc.compile()
    _NC_CACHE["nc"] = nc
    return nc


def kernel(feature, m, W_f, g_f, b_f, mu_f, v_f, W_o, g_o, b_o, mu_o, v_o):
    feature = np.asarray(feature, dtype=np.float32)
    m = np.asarray(m, dtype=np.float32)
    W_f = np.asarray(W_f, dtype=np.float32)
    W_o = np.asarray(W_o, dtype=np.float32)
    g_f, b_f, mu_f, v_f = (np.asarray(x, dtype=np.float32) for x in (g_f, b_f, mu_f, v_f))
    g_o, b_o, mu_o, v_o = (np.asarray(x, dtype=np.float32) for x in (g_o, b_o, mu_o, v_o))

    inv_f = g_f / np.sqrt(v_f + EPS)
    beta_f_v = b_f - mu_f * inv_f
    inv_o = g_o / np.sqrt(v_o + EPS)
    beta_o_v = b_o - mu_o * inv_o
    Wf_p = (inv_f[:, None] * W_f).astype(np.float32)          # [C, C]
    Wo1_p = (inv_o[:, None] * W_o[:, :C]).astype(np.float32)  # [C, C]
    Wo2_p = (inv_o[:, None] * W_o[:, C:]).astype(np.float32)  # [C, C]

    # lhsT layouts: blocks ci*2+o of W'^T
    def blocks_t(Wp):
        a = np.empty((P, 512), np.float32)
        for ci in range(2):
            for o in range(2):
                a[:, (ci * 2 + o) * P:(ci * 2 + o + 1) * P] = \
                    Wp[o * P:(o + 1) * P, ci * P:(ci + 1) * P].T
        return a

    wf_t_np = blocks_t(Wf_p)
    wo1_t_np = blocks_t(Wo1_p)
    wo2t_np = np.concatenate([Wo2_p.T[0:P, :], Wo2_p.T[P:C, :]], axis=1)  # [128, 512]
    beta_f_np = beta_f_v.reshape(2, P).T.copy()   # col ci = beta_f[ci*128:(ci+1)*128]
    beta_o_np = beta_o_v.reshape(2, P).T.copy()
    band_er, band_dl, cnt_er = _band_consts()

    nc = build()
    common = {
        "wf_t": wf_t_np, "wo1_t": wo1_t_np, "wo2t": wo2t_np,
        "beta_f": beta_f_np, "beta_o": beta_o_np,
        "band_er": band_er, "band_dl": band_dl, "cnt_er": cnt_er,
    }
    in_maps = []
    for b in range(B):
        im = dict(common)
        im["feature"] = np.ascontiguousarray(feature[b].reshape(C, HW))
        im["m"] = np.ascontiguousarray(m[b])
        in_maps.append(im)

    res = bass_utils.run_bass_kernel_spmd(nc, in_maps, list(range(B)))
    out = np.empty((B, C, H, W), np.float32)
    for b in range(B):
        out[b] = res.results[b]["out"].reshape(C, H, W)
    return out


# revision 7
# speedup vs baseline: 1.1785x; 1.0988x over previous
"""Trainium2 Bass kernel for nn_BDFM_Multi (B=8,C=256,H=W=128,N=4).

Data-parallel over batch: one batch element per NeuronCore (8 cores).

Per-core computation (feature f [C,HW], m [N,H,W], HW=16384):
  z    = (m > 0.3)                                  binary
  er   = 13-tap separable min-filter(z), dl = 13-tap separable max-filter(z)
         (composition of 4 iters of 4x4 cv2-style erode/dilate)
         -> computed as banded 0/1 matmuls + thresholds (exact on binary data)
  fbu  = per-class channels (er, 1-dl, dl-er)       [12, HW]
  mid  = fbu @ f^T                                  [12, C]
  A'   = Wo2' @ mid^T                               [C, 12]
  G    = A' @ mid                                   [C, C]
  cf   = Wf' @ f                                    [C, HW]   (BN-folded 1x1 conv)
  u    = G @ beta_f + beta_o                        [C]
  out  = Wo1' @ f + G @ cf + u                      [C, HW]
  (algebraic refactor of out = BN(Wo @ [f; mid^T @ (mid @ BN(Wf@f))]))

All big matmuls run in float32r (~2e-4 rel err, full PE rate).
"""
import numpy as np
from contextlib import ExitStack

import concourse.bass as bass
import concourse.mybir as mybir
import concourse.tile as tile
from concourse import bacc
from concourse import bass_utils
from concourse.masks import make_identity

F32 = mybir.dt.float32
F32R = mybir.dt.float32r
ALU = mybir.AluOpType
ACTF = mybir.ActivationFunctionType

B, C, H, W, N = 8, 256, 128, 128, 4
HW = H * W
EPS = 1e-5
P = 128
PT = 512              # p-tile width for pass 2
NPT = HW // PT        # 32 p-tiles
G1 = 4                # h-chunks per transpose group in pass 1

_NC_CACHE = {}


def _band_consts():
    idx = np.arange(P)
    # erosion: output i covers input [i-8, i+4]; dilation: [i-4, i+8]
    band_er = ((idx[:, None] >= idx[None, :] - 8) &
               (idx[:, None] <= idx[None, :] + 4)).astype(np.float32)
    band_dl = ((idx[:, None] >= idx[None, :] - 4) &
               (idx[:, None] <= idx[None, :] + 8)).astype(np.float32)
    cnt_er = band_er.sum(axis=0, dtype=np.float32).reshape(P, 1)
    return band_er, band_dl, cnt_er


def build():
    if "nc" in _NC_CACHE:
        return _NC_CACHE["nc"]
    nc = bacc.Bacc(trn_type="TRN2", target_bir_lowering=False, debug=False)

    feature = nc.dram_tensor("feature", [C, HW], F32, kind="ExternalInput")
    m_in = nc.dram_tensor("m", [N, H, W], F32, kind="ExternalInput")
    wf_t = nc.dram_tensor("wf_t", [P, 512], F32, kind="ExternalInput")    # blocks ci*2+o
    wo1_t = nc.dram_tensor("wo1_t", [P, 512], F32, kind="ExternalInput")  # blocks ci*2+o
    wo2t = nc.dram_tensor("wo2t", [P, 512], F32, kind="ExternalInput")    # blocks ci -> 256 cols
    beta_f = nc.dram_tensor("beta_f", [P, 2], F32, kind="ExternalInput")  # col ci
    beta_o = nc.dram_tensor("beta_o", [P, 2], F32, kind="ExternalInput")  # col o
    band_er = nc.dram_tensor("band_er", [P, P], F32, kind="ExternalInput")
    band_dl = nc.dram_tensor("band_dl", [P, P], F32, kind="ExternalInput")
    cnt_er = nc.dram_tensor("cnt_er", [P, 1], F32, kind="ExternalInput")
    out = nc.dram_tensor("out", [C, HW], F32, kind="ExternalOutput")

    with tile.TileContext(nc) as tc, ExitStack() as ctx:
        persist = ctx.enter_context(tc.tile_pool(name="persist", bufs=1))

        # ---------------- loads ----------------
        feat = persist.tile([P, 2 * HW], F32R)   # c-blk0 | c-blk1, cast fp32->fp32r

        m_sb = persist.tile([P, N * P], F32)
        for n in range(N):
            nc.sync.dma_start(out=m_sb[:, n * P:(n + 1) * P], in_=m_in[n])

        wf_sb = persist.tile([P, 512], F32R)
        nc.gpsimd.dma_start(out=wf_sb[:], in_=wf_t[:])
        wo1_sb = persist.tile([P, 512], F32R)
        nc.gpsimd.dma_start(out=wo1_sb[:], in_=wo1_t[:])
        wo2_sb = persist.tile([P, 512], F32R)
        nc.gpsimd.dma_start(out=wo2_sb[:], in_=wo2t[:])
        betaf_sb = persist.tile([P, 2], F32)
        nc.sync.dma_start(out=betaf_sb[:], in_=beta_f[:])
        betao_sb = persist.tile([P, 2], F32)
        nc.sync.dma_start(out=betao_sb[:], in_=beta_o[:])
        bander_sb = persist.tile([P, P], F32R)
        nc.gpsimd.dma_start(out=bander_sb[:], in_=band_er[:])
        banddl_sb = persist.tile([P, P], F32R)
        nc.gpsimd.dma_start(out=banddl_sb[:], in_=band_dl[:])
        cnt_sb = persist.tile([P, 1], F32)
        nc.sync.dma_start(out=cnt_sb[:], in_=cnt_er[:])

        # big feature load last on the SWDGE queue, blk-interleaved so pass-1
        # h-chunks unblock as early as possible
        for q in range(8):
            for blk in range(2):
                nc.gpsimd.dma_start(
                    out=feat[:, blk * HW + q * 2048: blk * HW + (q + 1) * 2048],
                    in_=feature[blk * P:(blk + 1) * P, q * 2048:(q + 1) * 2048])

        ident = persist.tile([P, P], F32)
        make_identity(nc, ident)
        ident_r = persist.tile([P, P], F32R)
        nc.vector.tensor_copy(ident_r[:], ident[:])

        # persistent results of phase 1
        fbuT = persist.tile([P, P * 12], F32R)     # [w, h*12 + k]
        gt_sb = persist.tile([P, 512], F32R)       # G^T blocks [c, ci*256 + o*128]
        gt_f = persist.tile([P, 512], F32)         # fp32 copy for the tiny u matmul
        u_sb = persist.tile([P, 2], F32)           # bias per o-blk

        # ---------------- morphology ----------------
        with tc.tile_pool(name="morph", bufs=1) as mo, \
             tc.tile_pool(name="morph_ps", bufs=2, space="PSUM") as mops:
            z_sb = mo.tile([P, N * P], F32R)
            nc.vector.tensor_scalar(z_sb[:], m_sb[:], 0.3, None, op0=ALU.is_gt)

            ps_rows_er = mops.tile([P, N * P], F32, tag="mps")
            nc.tensor.matmul(ps_rows_er[:], bander_sb[:], z_sb[:],
                             start=True, stop=True)
            ps_rows_dl = mops.tile([P, N * P], F32, tag="mps")
            nc.tensor.matmul(ps_rows_dl[:], banddl_sb[:], z_sb[:],
                             start=True, stop=True)

            rows_er = mo.tile([P, N * P], F32R)
            nc.vector.tensor_scalar(rows_er[:], ps_rows_er[:], cnt_sb[:, 0:1], None,
                                    op0=ALU.is_equal)
            rows_dl = mo.tile([P, N * P], F32R)
            nc.vector.tensor_scalar(rows_dl[:], ps_rows_dl[:], 0.5, None,
                                    op0=ALU.is_gt)

            # transpose each class tile -> [w, h]
            rows_erT = mo.tile([P, N * P], F32R)
            rows_dlT = mo.tile([P, N * P], F32R)
            for n in range(N):
                ps_tr = mops.tile([P, 2 * P], F32R, tag="mps")
                nc.tensor.matmul(ps_tr[:, 0:P], rows_er[:, n * P:(n + 1) * P],
                                 ident_r[:], is_transpose=True)
                nc.tensor.matmul(ps_tr[:, P:2 * P], rows_dl[:, n * P:(n + 1) * P],
                                 ident_r[:], is_transpose=True)
                nc.vector.tensor_copy(rows_erT[:, n * P:(n + 1) * P], ps_tr[:, 0:P])
                nc.vector.tensor_copy(rows_dlT[:, n * P:(n + 1) * P], ps_tr[:, P:2 * P])

            ps_cols_er = mops.tile([P, N * P], F32, tag="mps")
            nc.tensor.matmul(ps_cols_er[:], bander_sb[:], rows_erT[:],
                             start=True, stop=True)
            ps_cols_dl = mops.tile([P, N * P], F32, tag="mps")
            nc.tensor.matmul(ps_cols_dl[:], banddl_sb[:], rows_dlT[:],
                             start=True, stop=True)

            er_t = mo.tile([P, N * P], F32)   # er^T per class [w, h]
            dl_t = mo.tile([P, N * P], F32)
            nc.vector.tensor_scalar(er_t[:], ps_cols_er[:], cnt_sb[:, 0:1], None,
                                    op0=ALU.is_equal)
            nc.vector.tensor_scalar(dl_t[:], ps_cols_dl[:], 0.5, None, op0=ALU.is_gt)

            # write channels into fbuT at [w, h*12 + k], k = 3n+j
            fbuT_v = fbuT.rearrange("w (h k) -> w h k", k=12)
            for n in range(N):
                src_er = er_t[:, n * P:(n + 1) * P]
                src_dl = dl_t[:, n * P:(n + 1) * P]
                nc.vector.tensor_copy(fbuT_v[:, :, 3 * n], src_er)
                nc.vector.tensor_scalar(fbuT_v[:, :, 3 * n + 1], src_dl, 0.0, None,
                                        op0=ALU.is_equal)
                nc.vector.tensor_tensor(fbuT_v[:, :, 3 * n + 2], src_dl, src_er,
                                        op=ALU.subtract)

        # ---------------- pass 1: feature transpose + mid ----------------
        mid_r = persist.tile([12, 256], F32R)
        with tc.tile_pool(name="mid_ps", bufs=1, space="PSUM") as midps, \
             tc.tile_pool(name="p1_ps", bufs=2, space="PSUM") as p1ps, \
             tc.tile_pool(name="p1_sb", bufs=3) as p1sb:
            mid_ps = midps.tile([12, 256], F32)
            for g in range(P // G1):
                tr = p1ps.tile([P, G1 * 256], F32R, tag="tr")
                for j in range(G1):
                    h = g * G1 + j
                    nc.tensor.matmul(tr[:, j * 256:j * 256 + P],
                                     feat[:, h * P:(h + 1) * P],
                                     ident_r[:], is_transpose=True)
                    nc.tensor.matmul(tr[:, j * 256 + P:(j + 1) * 256],
                                     feat[:, HW + h * P:HW + (h + 1) * P],
                                     ident_r[:], is_transpose=True)
                ft = p1sb.tile([P, G1 * 256], F32R, tag="ft")
                if g % 2 == 0:
                    nc.vector.tensor_copy(ft[:], tr[:])
                else:
                    nc.scalar.copy(ft[:], tr[:])
                for j in range(G1):
                    h = g * G1 + j
                    nc.tensor.matmul(mid_ps[:], fbuT[:, h * 12:h * 12 + 12],
                                     ft[:, j * 256:(j + 1) * 256],
                                     start=(h == 0), stop=(h == P - 1),
                                     skip_group_check=True)
            nc.vector.tensor_copy(mid_r[:], mid_ps[:])

        # ---------------- small stage: mid^T, A'^T, G^T, u ----------------
        with tc.tile_pool(name="sm_ps", bufs=1, space="PSUM") as smps, \
             tc.tile_pool(name="sm_sb", bufs=1) as smsb:
            # mid^T via PE transpose of [12,128] chunks (fp32r)
            ps_mt = smps.tile([P, 24], F32R, tag="mt")
            for ci in range(2):
                nc.tensor.matmul(ps_mt[:, ci * 12:(ci + 1) * 12],
                                 mid_r[:, ci * P:(ci + 1) * P],
                                 ident_r[0:12, 0:12], is_transpose=True)
            mid_t = smsb.tile([P, 24], F32R)
            nc.vector.tensor_copy(mid_t[:], ps_mt[:])

            # A'^T = mid @ Wo2'^T   [12, 256]
            ps_at = smps.tile([12, 256], F32, tag="at")
            nc.tensor.matmul(ps_at[:], mid_t[:, 0:12], wo2_sb[:, 0:256],
                             start=True, stop=False)
            nc.tensor.matmul(ps_at[:], mid_t[:, 12:24], wo2_sb[:, 256:512],
                             start=False, stop=True)
            a_t = smsb.tile([12, 256], F32R)
            nc.vector.tensor_copy(a_t[:], ps_at[:])

            # G^T[c, o] = sum_k mid[k, c] A'^T[k, o];  chunks ci on partitions
            ps_gt = smps.tile([P, 512], F32, tag="gt")
            for ci in range(2):
                nc.tensor.matmul(ps_gt[:, ci * 256:(ci + 1) * 256],
                                 mid_r[:, ci * P:(ci + 1) * P], a_t[:],
                                 start=True, stop=True)
            nc.vector.tensor_copy(gt_sb[:], ps_gt[:])
            nc.vector.tensor_copy(gt_f[:], ps_gt[:])

            # u = G @ beta_f + beta_o   per o-blk  (fp32 matmuls)
            for o in range(2):
                ps_u = smps.tile([P, 1], F32, tag="u")
                nc.tensor.matmul(ps_u[:], gt_f[:, o * P:(o + 1) * P],
                                 betaf_sb[:, 0:1], start=True, stop=False)
                nc.tensor.matmul(ps_u[:], gt_f[:, 256 + o * P:256 + (o + 1) * P],
                                 betaf_sb[:, 1:2], start=False, stop=True)
                nc.scalar.activation(u_sb[:, o:o + 1], ps_u[:], ACTF.Identity,
                                     bias=betao_sb[:, o:o + 1])

        # ---------------- pass 2: cf, out ----------------
        with tc.tile_pool(name="cf_ps", bufs=2, space="PSUM") as cfps, \
             tc.tile_pool(name="out_ps", bufs=2, space="PSUM") as outps, \
             tc.tile_pool(name="p2_sb", bufs=2) as p2sb:
            for t in range(NPT):
                c0 = t * PT
                cf_ps = cfps.tile([P, 2 * PT], F32, tag="cf")
                for o in range(2):
                    nc.tensor.matmul(cf_ps[:, o * PT:(o + 1) * PT],
                                     wf_sb[:, (0 * 2 + o) * P:(0 * 2 + o + 1) * P],
                                     feat[:, c0:c0 + PT],
                                     start=True, stop=False, skip_group_check=True)
                    nc.tensor.matmul(cf_ps[:, o * PT:(o + 1) * PT],
                                     wf_sb[:, (1 * 2 + o) * P:(1 * 2 + o + 1) * P],
                                     feat[:, HW + c0:HW + c0 + PT],
                                     start=False, stop=True, skip_group_check=True)
                cf = p2sb.tile([P, 2 * PT], F32R, tag="cf_sb")
                if t % 2 == 0:
                    nc.scalar.copy(cf[:], cf_ps[:])
                else:
                    nc.vector.tensor_copy(cf[:], cf_ps[:])

                ot = p2sb.tile([P, 2 * PT], F32, tag="ot")
                out_ps = outps.tile([P, 2 * PT], F32, tag="ops")
                for o in range(2):
                    ops = out_ps[:, o * PT:(o + 1) * PT]
                    nc.tensor.matmul(ops,
                                     wo1_sb[:, (0 * 2 + o) * P:(0 * 2 + o + 1) * P],
                                     feat[:, c0:c0 + PT],
                                     start=True, stop=False, skip_group_check=True)
                    nc.tensor.matmul(ops,
                                     wo1_sb[:, (1 * 2 + o) * P:(1 * 2 + o + 1) * P],
                                     feat[:, HW + c0:HW + c0 + PT],
                                     start=False, stop=False, skip_group_check=True)
                    nc.tensor.matmul(ops,
                                     gt_sb[:, o * P:(o + 1) * P],
                                     cf[:, 0:PT],
                                     start=False, stop=False, skip_group_check=True)
                    nc.tensor.matmul(ops,
                                     gt_sb[:, 256 + o * P:256 + (o + 1) * P],
                                     cf[:, PT:2 * PT],
                                     start=False, stop=True, skip_group_check=True)
                    if o == 0:
                        nc.scalar.activation(ot[:, o * PT:(o + 1) * PT], ops,
                                             ACTF.Identity, bias=u_sb[:, o:o + 1])
                    else:
                        nc.vector.tensor_scalar(ot[:, o * PT:(o + 1) * PT], ops,
                                                u_sb[:, o:o + 1], None, op0=ALU.add)
                nc.sync.dma_start(out=out[0:P, c0:c0 + PT], in_=ot[:, 0:PT])
                nc.sync.dma_start(out=out[P:C, c0:c0 + PT], in_=ot[:, PT:2 * PT])

    nc.compile()
    _NC_CACHE["nc"] = nc
    return nc


def kernel(feature, m, W_f, g_f, b_f, mu_f, v_f, W_o, g_o, b_o, mu_o, v_o):
    feature = np.asarray(feature, dtype=np.float32)
    m = np.asarray(m, dtype=np.float32)
    W_f = np.asarray(W_f, dtype=np.float32)
    W_o = np.asarray(W_o, dtype=np.float32)
    g_f, b_f, mu_f, v_f = (np.asarray(x, dtype=np.float32) for x in (g_f, b_f, mu_f, v_f))
    g_o, b_o, mu_o, v_o = (np.asarray(x, dtype=np.float32) for x in (g_o, b_o, mu_o, v_o))

    inv_f = g_f / np.sqrt(v_f + EPS)
    beta_f_v = b_f - mu_f * inv_f
    inv_o = g_o / np.sqrt(v_o + EPS)
    beta_o_v = b_o - mu_o * inv_o
    Wf_p = (inv_f[:, None] * W_f).astype(np.float32)          # [C, C]
    Wo1_p = (inv_o[:, None] * W_o[:, :C]).astype(np.float32)  # [C, C]
    Wo2_p = (inv_o[:, None] * W_o[:, C:]).astype(np.float32)  # [C, C]

    # lhsT layouts: blocks ci*2+o of W'^T
    def blocks_t(Wp):
        a = np.empty((P, 512), np.float32)
        for ci in range(2):
            for o in range(2):
                a[:, (ci * 2 + o) * P:(ci * 2 + o + 1) * P] = \
                    Wp[o * P:(o + 1) * P, ci * P:(ci + 1) * P].T
        return a

    wf_t_np = blocks_t(Wf_p)
    wo1_t_np = blocks_t(Wo1_p)
    wo2t_np = np.concatenate([Wo2_p.T[0:P, :], Wo2_p.T[P:C, :]], axis=1)  # [128, 512]
    beta_f_np = beta_f_v.reshape(2, P).T.copy()   # col ci = beta_f[ci*128:(ci+1)*128]
    beta_o_np = beta_o_v.reshape(2, P).T.copy()
    band_er, band_dl, cnt_er = _band_consts()

    nc = build()
    common = {
        "wf_t": wf_t_np, "wo1_t": wo1_t_np, "wo2t": wo2t_np,
        "beta_f": beta_f_np, "beta_o": beta_o_np,
        "band_er": band_er, "band_dl": band_dl, "cnt_er": cnt_er,
    }
    in_maps = []
    for b in range(B):
        im = dict(common)
        im["feature"] = np.ascontiguousarray(feature[b].reshape(C, HW))
        im["m"] = np.ascontiguousarray(m[b])
        in_maps.append(im)

    res = bass_utils.run_bass_kernel_spmd(nc, in_maps, list(range(B)))
    out = np.empty((B, C, H, W), np.float32)
    for b in range(B):
        out[b] = res.results[b]["out"].reshape(C, H, W)
    return out


# revision 8
# speedup vs baseline: 1.4021x; 1.1897x over previous
"""Trainium2 Bass kernel for nn_BDFM_Multi (B=8,C=256,H=W=128,N=4).

Data-parallel over batch: one batch element per NeuronCore (8 cores).

Per-core computation (feature f [C,HW], m [N,H,W], HW=16384):
  z    = (m > 0.3)                                  binary
  er   = 13-tap separable min-filter(z), dl = 13-tap separable max-filter(z)
         (composition of 4 iters of 4x4 cv2-style erode/dilate)
         -> computed as banded 0/1 matmuls + thresholds (exact on binary data)
  fbu  = per-class channels (er, 1-dl, dl-er)       [12, HW]
  mid  = fbu @ f^T                                  [12, C]
  A'   = Wo2' @ mid^T                               [C, 12]
  G    = A' @ mid                                   [C, C]
  Wc   = Wo1' + G @ Wf'                             [C, C]  <- key collapse:
         out = Wo1'@f + G@(Wf'@f + beta_f 1^T) + beta_o 1^T
             = Wc @ f + u 1^T,   u = G @ beta_f + beta_o
  out  = Wc @ f + u                                 [C, HW]
  (exact algebraic refactor of out = BN(Wo @ [f; mid^T @ (mid @ BN(Wf@f))]))

All big matmuls run in float32r (~2-4e-4 rel err, full PE rate).
"""
import numpy as np
from contextlib import ExitStack

import concourse.bass as bass
import concourse.mybir as mybir
import concourse.tile as tile
from concourse import bacc
from concourse import bass_utils
from concourse.masks import make_identity

F32 = mybir.dt.float32
F32R = mybir.dt.float32r
ALU = mybir.AluOpType
ACTF = mybir.ActivationFunctionType

B, C, H, W, N = 8, 256, 128, 128, 4
HW = H * W
EPS = 1e-5
P = 128
PT = 512              # p-tile width for pass 2
NPT = HW // PT        # 32 p-tiles
G1 = 4                # h-chunks per transpose group in pass 1

_NC_CACHE = {}


def _band_consts():
    idx = np.arange(P)
    # erosion: output i covers input [i-8, i+4]; dilation: [i-4, i+8]
    band_er = ((idx[:, None] >= idx[None, :] - 8) &
               (idx[:, None] <= idx[None, :] + 4)).astype(np.float32)
    band_dl = ((idx[:, None] >= idx[None, :] - 4) &
               (idx[:, None] <= idx[None, :] + 8)).astype(np.float32)
    cnt_er = band_er.sum(axis=0, dtype=np.float32).reshape(P, 1)
    return band_er, band_dl, cnt_er


# layout of the packed fp32r parameter tensor "pk" [128, 1280]:
#   [0:512)     wo2t   : Wo2'^T chunks   [c-chunk ci -> cols ci*256:(ci+1)*256]
#   [512:1024)  wf_n   : Wf' blocks      [(ci*2+a)*128 ...] = Wf'[ci*128:,a*128:]
#   [1024:1152) band_er
#   [1152:1280) band_dl
PK_W = 1280


def build():
    if "nc" in _NC_CACHE:
        return _NC_CACHE["nc"]
    nc = bacc.Bacc(trn_type="TRN2", target_bir_lowering=False, debug=False)

    feature = nc.dram_tensor("feature", [C, HW], F32, kind="ExternalInput")
    m_in = nc.dram_tensor("m", [N, H, W], F32, kind="ExternalInput")
    pk = nc.dram_tensor("pk", [P, PK_W], F32, kind="ExternalInput")
    wo1_t = nc.dram_tensor("wo1_t", [P, 512], F32, kind="ExternalInput")  # blocks ci*2+o
    beta_f = nc.dram_tensor("beta_f", [P, 2], F32, kind="ExternalInput")  # col ci
    beta_o = nc.dram_tensor("beta_o", [P, 2], F32, kind="ExternalInput")  # col o
    cnt_er = nc.dram_tensor("cnt_er", [P, 1], F32, kind="ExternalInput")
    out = nc.dram_tensor("out", [C, HW], F32, kind="ExternalOutput")

    with tile.TileContext(nc) as tc, ExitStack() as ctx:
        persist = ctx.enter_context(tc.tile_pool(name="persist", bufs=1))

        # ---------------- loads ----------------
        # single fp32r cast-load for all packed params (first on the SWDGE queue)
        pk_sb = persist.tile([P, PK_W], F32R)
        nc.gpsimd.dma_start(out=pk_sb[:], in_=pk[:])
        wo2_sb = pk_sb[:, 0:512]
        wfn_sb = pk_sb[:, 512:1024]
        bander_sb = pk_sb[:, 1024:1152]
        banddl_sb = pk_sb[:, 1152:1280]

        m_sb = persist.tile([P, N * P], F32)
        for n in range(N):
            nc.sync.dma_start(out=m_sb[:, n * P:(n + 1) * P], in_=m_in[n])
        wo1_sb = persist.tile([P, 512], F32)
        nc.sync.dma_start(out=wo1_sb[:], in_=wo1_t[:])
        betaf_sb = persist.tile([P, 2], F32)
        nc.sync.dma_start(out=betaf_sb[:], in_=beta_f[:])
        betao_sb = persist.tile([P, 2], F32)
        nc.sync.dma_start(out=betao_sb[:], in_=beta_o[:])
        cnt_sb = persist.tile([P, 1], F32)
        nc.sync.dma_start(out=cnt_sb[:], in_=cnt_er[:])

        # big feature load on the SWDGE queue, blk-interleaved so pass-1
        # h-chunks unblock as early as possible
        feat = persist.tile([P, 2 * HW], F32R)   # c-blk0 | c-blk1, cast fp32->fp32r
        for q in range(8):
            for blk in range(2):
                nc.gpsimd.dma_start(
                    out=feat[:, blk * HW + q * 2048: blk * HW + (q + 1) * 2048],
                    in_=feature[blk * P:(blk + 1) * P, q * 2048:(q + 1) * 2048])

        ident = persist.tile([P, P], F32)
        make_identity(nc, ident)
        ident_r = persist.tile([P, P], F32R)
        nc.vector.tensor_copy(ident_r[:], ident[:])

        # persistent results of phase 1
        fbuT = persist.tile([P, P * 12], F32R)     # [w, h*12 + k]
        wc_sb = persist.tile([P, 512], F32R)       # Wc^T blocks (a*2+o)
        u_sb = persist.tile([P, 2], F32)           # bias per o-blk

        # ---------------- morphology ----------------
        with tc.tile_pool(name="morph", bufs=1) as mo, \
             tc.tile_pool(name="morph_ps", bufs=2, space="PSUM") as mops:
            z_sb = mo.tile([P, N * P], F32R)
            nc.vector.tensor_scalar(z_sb[:], m_sb[:], 0.3, None, op0=ALU.is_gt)

            ps_rows_er = mops.tile([P, N * P], F32, tag="mps")
            nc.tensor.matmul(ps_rows_er[:], bander_sb, z_sb[:],
                             start=True, stop=True)
            ps_rows_dl = mops.tile([P, N * P], F32, tag="mps")
            nc.tensor.matmul(ps_rows_dl[:], banddl_sb, z_sb[:],
                             start=True, stop=True)

            rows_er = mo.tile([P, N * P], F32R)
            nc.vector.tensor_scalar(rows_er[:], ps_rows_er[:], cnt_sb[:, 0:1], None,
                                    op0=ALU.is_equal)
            rows_dl = mo.tile([P, N * P], F32R)
            nc.vector.tensor_scalar(rows_dl[:], ps_rows_dl[:], 0.5, None,
                                    op0=ALU.is_gt)

            # transpose each class tile -> [w, h]
            rows_erT = mo.tile([P, N * P], F32R)
            rows_dlT = mo.tile([P, N * P], F32R)
            for n in range(N):
                ps_tr = mops.tile([P, 2 * P], F32R, tag="mps")
                nc.tensor.matmul(ps_tr[:, 0:P], rows_er[:, n * P:(n + 1) * P],
                                 ident_r[:], is_transpose=True)
                nc.tensor.matmul(ps_tr[:, P:2 * P], rows_dl[:, n * P:(n + 1) * P],
                                 ident_r[:], is_transpose=True)
                nc.vector.tensor_copy(rows_erT[:, n * P:(n + 1) * P], ps_tr[:, 0:P])
                nc.vector.tensor_copy(rows_dlT[:, n * P:(n + 1) * P], ps_tr[:, P:2 * P])

            ps_cols_er = mops.tile([P, N * P], F32, tag="mps")
            nc.tensor.matmul(ps_cols_er[:], bander_sb, rows_erT[:],
                             start=True, stop=True)
            ps_cols_dl = mops.tile([P, N * P], F32, tag="mps")
            nc.tensor.matmul(ps_cols_dl[:], banddl_sb, rows_dlT[:],
                             start=True, stop=True)

            er_t = mo.tile([P, N * P], F32)   # er^T per class [w, h]
            dl_t = mo.tile([P, N * P], F32)
            nc.vector.tensor_scalar(er_t[:], ps_cols_er[:], cnt_sb[:, 0:1], None,
                                    op0=ALU.is_equal)
            nc.vector.tensor_scalar(dl_t[:], ps_cols_dl[:], 0.5, None, op0=ALU.is_gt)

            # write channels into fbuT at [w, h*12 + k], k = 3n+j
            fbuT_v = fbuT.rearrange("w (h k) -> w h k", k=12)
            for n in range(N):
                src_er = er_t[:, n * P:(n + 1) * P]
                src_dl = dl_t[:, n * P:(n + 1) * P]
                nc.vector.tensor_copy(fbuT_v[:, :, 3 * n], src_er)
                nc.vector.tensor_scalar(fbuT_v[:, :, 3 * n + 1], src_dl, 0.0, None,
                                        op0=ALU.is_equal)
                nc.vector.tensor_tensor(fbuT_v[:, :, 3 * n + 2], src_dl, src_er,
                                        op=ALU.subtract)

        # ---------------- pass 1: feature transpose + mid ----------------
        mid_r = persist.tile([12, 256], F32R)
        with tc.tile_pool(name="mid_ps", bufs=1, space="PSUM") as midps, \
             tc.tile_pool(name="p1_ps", bufs=2, space="PSUM") as p1ps, \
             tc.tile_pool(name="p1_sb", bufs=3) as p1sb:
            mid_ps = midps.tile([12, 256], F32)
            for g in range(P // G1):
                tr = p1ps.tile([P, G1 * 256], F32R, tag="tr")
                for j in range(G1):
                    h = g * G1 + j
                    nc.tensor.matmul(tr[:, j * 256:j * 256 + P],
                                     feat[:, h * P:(h + 1) * P],
                                     ident_r[:], is_transpose=True)
                    nc.tensor.matmul(tr[:, j * 256 + P:(j + 1) * 256],
                                     feat[:, HW + h * P:HW + (h + 1) * P],
                                     ident_r[:], is_transpose=True)
                ft = p1sb.tile([P, G1 * 256], F32R, tag="ft")
                if g % 2 == 0:
                    nc.vector.tensor_copy(ft[:], tr[:])
                else:
                    nc.scalar.copy(ft[:], tr[:])
                for j in range(G1):
                    h = g * G1 + j
                    nc.tensor.matmul(mid_ps[:], fbuT[:, h * 12:h * 12 + 12],
                                     ft[:, j * 256:(j + 1) * 256],
                                     start=(h == 0), stop=(h == P - 1),
                                     skip_group_check=True)
            nc.vector.tensor_copy(mid_r[:], mid_ps[:])

        # ---------------- small stage: mid^T, A'^T, G^T, Wc, u ----------------
        with tc.tile_pool(name="sm_ps", bufs=1, space="PSUM") as smps, \
             tc.tile_pool(name="sm_sb", bufs=1) as smsb:
            # mid^T via PE transpose of [12,128] chunks (fp32r)
            ps_mt = smps.tile([P, 24], F32R, tag="mt")
            for ci in range(2):
                nc.tensor.matmul(ps_mt[:, ci * 12:(ci + 1) * 12],
                                 mid_r[:, ci * P:(ci + 1) * P],
                                 ident_r[0:12, 0:12], is_transpose=True)
            mid_t = smsb.tile([P, 24], F32R)
            nc.vector.tensor_copy(mid_t[:], ps_mt[:])

            # A'^T = mid @ Wo2'^T   [12, 256]
            ps_at = smps.tile([12, 256], F32, tag="at")
            nc.tensor.matmul(ps_at[:], mid_t[:, 0:12], wo2_sb[:, 0:256],
                             start=True, stop=False)
            nc.tensor.matmul(ps_at[:], mid_t[:, 12:24], wo2_sb[:, 256:512],
                             start=False, stop=True)
            a_t = smsb.tile([12, 256], F32R)
            nc.vector.tensor_copy(a_t[:], ps_at[:])

            # G^T[c, o] = sum_k mid[k, c] A'^T[k, o];  chunks ci on partitions
            ps_gt = smps.tile([P, 512], F32, tag="gt")
            for ci in range(2):
                nc.tensor.matmul(ps_gt[:, ci * 256:(ci + 1) * 256],
                                 mid_r[:, ci * P:(ci + 1) * P], a_t[:],
                                 start=True, stop=True)
            gt_r = smsb.tile([P, 512], F32R)
            nc.vector.tensor_copy(gt_r[:], ps_gt[:])
            gt_f = smsb.tile([P, 512], F32)
            nc.vector.tensor_copy(gt_f[:], ps_gt[:])

            # X = Wf'^T @ G^T (= (G Wf')^T); blocks a (c_in chunk) on partitions
            ps_x = smps.tile([P, 512], F32, tag="x")
            for a in range(2):
                for ci in range(2):
                    nc.tensor.matmul(ps_x[:, a * 256:(a + 1) * 256],
                                     wfn_sb[:, (ci * 2 + a) * P:(ci * 2 + a + 1) * P],
                                     gt_r[:, ci * 256:(ci + 1) * 256],
                                     start=(ci == 0), stop=(ci == 1),
                                     skip_group_check=True)
            # Wc^T = Wo1'^T + X  (blocks (a*2+o) align with [a*256 + o*128])
            for a in range(2):
                nc.vector.tensor_tensor(wc_sb[:, a * 256:(a + 1) * 256],
                                        ps_x[:, a * 256:(a + 1) * 256],
                                        wo1_sb[:, a * 256:(a + 1) * 256],
                                        op=ALU.add)

            # u = G @ beta_f + beta_o   per o-blk  (fp32 matmuls)
            for o in range(2):
                ps_u = smps.tile([P, 1], F32, tag="u")
                nc.tensor.matmul(ps_u[:], gt_f[:, o * P:(o + 1) * P],
                                 betaf_sb[:, 0:1], start=True, stop=False)
                nc.tensor.matmul(ps_u[:], gt_f[:, 256 + o * P:256 + (o + 1) * P],
                                 betaf_sb[:, 1:2], start=False, stop=True)
                nc.scalar.activation(u_sb[:, o:o + 1], ps_u[:], ACTF.Identity,
                                     bias=betao_sb[:, o:o + 1])

        # ---------------- pass 2: out = Wc @ f + u ----------------
        with tc.tile_pool(name="out_ps", bufs=4, space="PSUM") as outps, \
             tc.tile_pool(name="p2_sb", bufs=3) as p2sb:
            for t in range(NPT):
                c0 = t * PT
                ot = p2sb.tile([P, 2 * PT], F32, tag="ot")
                out_ps = outps.tile([P, 2 * PT], F32, tag="ops")
                for o in range(2):
                    ops = out_ps[:, o * PT:(o + 1) * PT]
                    nc.tensor.matmul(ops,
                                     wc_sb[:, (0 * 2 + o) * P:(0 * 2 + o + 1) * P],
                                     feat[:, c0:c0 + PT],
                                     start=True, stop=False, skip_group_check=True)
                    nc.tensor.matmul(ops,
                                     wc_sb[:, (1 * 2 + o) * P:(1 * 2 + o + 1) * P],
                                     feat[:, HW + c0:HW + c0 + PT],
                                     start=False, stop=True, skip_group_check=True)
                    if o == 0:
                        nc.scalar.activation(ot[:, o * PT:(o + 1) * PT], ops,
                                             ACTF.Identity, bias=u_sb[:, o:o + 1])
                    else:
                        nc.vector.tensor_scalar(ot[:, o * PT:(o + 1) * PT], ops,
                                                u_sb[:, o:o + 1], None, op0=ALU.add)
                nc.sync.dma_start(out=out[0:P, c0:c0 + PT], in_=ot[:, 0:PT])
                nc.sync.dma_start(out=out[P:C, c0:c0 + PT], in_=ot[:, PT:2 * PT])

    nc.compile()
    _NC_CACHE["nc"] = nc
    return nc


def prepare_in_maps(feature, m, W_f, g_f, b_f, mu_f, v_f, W_o, g_o, b_o, mu_o, v_o):
    feature = np.asarray(feature, dtype=np.float32)
    m = np.asarray(m, dtype=np.float32)
    W_f = np.asarray(W_f, dtype=np.float32)
    W_o = np.asarray(W_o, dtype=np.float32)
    g_f, b_f, mu_f, v_f = (np.asarray(x, dtype=np.float32) for x in (g_f, b_f, mu_f, v_f))
    g_o, b_o, mu_o, v_o = (np.asarray(x, dtype=np.float32) for x in (g_o, b_o, mu_o, v_o))

    inv_f = g_f / np.sqrt(v_f + EPS)
    beta_f_v = b_f - mu_f * inv_f
    inv_o = g_o / np.sqrt(v_o + EPS)
    beta_o_v = b_o - mu_o * inv_o
    Wf_p = (inv_f[:, None] * W_f).astype(np.float32)          # [C, C]
    Wo1_p = (inv_o[:, None] * W_o[:, :C]).astype(np.float32)  # [C, C]
    Wo2_p = (inv_o[:, None] * W_o[:, C:]).astype(np.float32)  # [C, C]

    def blocks_t(Wp):
        # lhsT layout: blocks ci*2+o of Wp^T
        a = np.empty((P, 512), np.float32)
        for ci in range(2):
            for o in range(2):
                a[:, (ci * 2 + o) * P:(ci * 2 + o + 1) * P] = \
                    Wp[o * P:(o + 1) * P, ci * P:(ci + 1) * P].T
        return a

    def blocks_n(Wp):
        # natural-layout blocks ci*2+a: Wp[ci*128:(ci+1)*128, a*128:(a+1)*128]
        a_ = np.empty((P, 512), np.float32)
        for ci in range(2):
            for a in range(2):
                a_[:, (ci * 2 + a) * P:(ci * 2 + a + 1) * P] = \
                    Wp[ci * P:(ci + 1) * P, a * P:(a + 1) * P]
        return a_

    band_er, band_dl, cnt_er = _band_consts()
    pk = np.empty((P, PK_W), np.float32)
    pk[:, 0:512] = np.concatenate([Wo2_p.T[0:P, :], Wo2_p.T[P:C, :]], axis=1)
    pk[:, 512:1024] = blocks_n(Wf_p)
    pk[:, 1024:1152] = band_er
    pk[:, 1152:1280] = band_dl

    common = {
        "pk": pk,
        "wo1_t": blocks_t(Wo1_p),
        "beta_f": beta_f_v.reshape(2, P).T.copy(),
        "beta_o": beta_o_v.reshape(2, P).T.copy(),
        "cnt_er": cnt_er,
    }
    in_maps = []
    for b in range(B):
        im = dict(common)
        im["feature"] = np.ascontiguousarray(feature[b].reshape(C, HW))
        im["m"] = np.ascontiguousarray(m[b])
        in_maps.append(im)
    return in_maps


def kernel(feature, m, W_f, g_f, b_f, mu_f, v_f, W_o, g_o, b_o, mu_o, v_o):
    nc = build()
    in_maps = prepare_in_maps(feature, m, W_f, g_f, b_f, mu_f, v_f,
                              W_o, g_o, b_o, mu_o, v_o)
    res = bass_utils.run_bass_kernel_spmd(nc, in_maps, list(range(B)))
    out = np.empty((B, C, H, W), np.float32)
    for b in range(B):
        out[b] = res.results[b]["out"].reshape(C, H, W)
    return out


# revision 9
# speedup vs baseline: 1.5081x; 1.0756x over previous
"""Trainium2 Bass kernel for nn_BDFM_Multi (B=8,C=256,H=W=128,N=4).

Data-parallel over batch: one batch element per NeuronCore (8 cores).

Per-core computation (feature f [C,HW], m [N,H,W], HW=16384):
  z    = (m > 0.3)                                  binary
  er   = 13-tap separable min-filter(z), dl = 13-tap separable max-filter(z)
         (composition of 4 iters of 4x4 cv2-style erode/dilate)
         -> computed as banded 0/1 matmuls + thresholds (exact on binary data)
  fbu  = per-class channels (er, 1-dl, dl-er)       [12, HW]
  mid  = fbu @ f^T                                  [12, C]
  A'   = Wo2' @ mid^T                               [C, 12]
  G    = A' @ mid                                   [C, C]
  Wc   = Wo1' + G @ Wf'                             [C, C]  <- key collapse:
         out = Wo1'@f + G@(Wf'@f + beta_f 1^T) + beta_o 1^T
             = Wc @ f + u 1^T,   u = G @ beta_f + beta_o
  out  = Wc @ f + u                                 [C, HW]
  (exact algebraic refactor of out = BN(Wo @ [f; mid^T @ (mid @ BN(Wf@f))]))

All big matmuls run in float32r (~2-4e-4 rel err, full PE rate).
"""
import numpy as np
from contextlib import ExitStack

import concourse.bass as bass
import concourse.mybir as mybir
import concourse.tile as tile
from concourse import bacc
from concourse import bass_utils
from concourse.masks import make_identity

F32 = mybir.dt.float32
F32R = mybir.dt.float32r
ALU = mybir.AluOpType
ACTF = mybir.ActivationFunctionType

B, C, H, W, N = 8, 256, 128, 128, 4
HW = H * W
EPS = 1e-5
P = 128
PT = 512              # p-tile width for pass 2
NPT = HW // PT        # 32 p-tiles
G1 = 4                # h-chunks per transpose group in pass 1

_NC_CACHE = {}


def _band_consts():
    idx = np.arange(P)
    # erosion: output i covers input [i-8, i+4]; dilation: [i-4, i+8]
    band_er = ((idx[:, None] >= idx[None, :] - 8) &
               (idx[:, None] <= idx[None, :] + 4)).astype(np.float32)
    band_dl = ((idx[:, None] >= idx[None, :] - 4) &
               (idx[:, None] <= idx[None, :] + 8)).astype(np.float32)
    cnt_er = band_er.sum(axis=0, dtype=np.float32).reshape(P, 1)
    return band_er, band_dl, cnt_er


# layout of the packed fp32r parameter tensor "pk" [128, 1280]:
#   [0:512)     wo2t   : Wo2'^T chunks   [c-chunk ci -> cols ci*256:(ci+1)*256]
#   [512:1024)  wf_n   : Wf' blocks      [(ci*2+a)*128 ...] = Wf'[ci*128:,a*128:]
#   [1024:1152) band_er
#   [1152:1280) band_dl
#   [1280:1408) identity
PK_W = 1408


def build():
    if "nc" in _NC_CACHE:
        return _NC_CACHE["nc"]
    nc = bacc.Bacc(trn_type="TRN2", target_bir_lowering=False, debug=False)

    feature = nc.dram_tensor("feature", [C, HW], F32, kind="ExternalInput")
    m_in = nc.dram_tensor("m", [N, H, W], F32, kind="ExternalInput")
    pk = nc.dram_tensor("pk", [P, PK_W], F32, kind="ExternalInput")
    wo1_t = nc.dram_tensor("wo1_t", [P, 512], F32, kind="ExternalInput")  # blocks ci*2+o
    beta_f = nc.dram_tensor("beta_f", [P, 2], F32, kind="ExternalInput")  # col ci
    beta_o = nc.dram_tensor("beta_o", [P, 2], F32, kind="ExternalInput")  # col o
    cnt_er = nc.dram_tensor("cnt_er", [P, 1], F32, kind="ExternalInput")
    out = nc.dram_tensor("out", [C, HW], F32, kind="ExternalOutput")

    with tile.TileContext(nc) as tc, ExitStack() as ctx:
        persist = ctx.enter_context(tc.tile_pool(name="persist", bufs=1))

        # ---------------- loads ----------------
        # single fp32r cast-load for all packed params (first on the SWDGE queue)
        pk_sb = persist.tile([P, PK_W], F32R)
        nc.gpsimd.dma_start(out=pk_sb[:], in_=pk[:])
        wo2_sb = pk_sb[:, 0:512]
        wfn_sb = pk_sb[:, 512:1024]
        bander_sb = pk_sb[:, 1024:1152]
        banddl_sb = pk_sb[:, 1152:1280]
        ident_r = pk_sb[:, 1280:1408]

        m_sb = persist.tile([P, N * P], F32)
        for n in range(N):
            nc.sync.dma_start(out=m_sb[:, n * P:(n + 1) * P], in_=m_in[n])
        wo1_sb = persist.tile([P, 512], F32)
        nc.sync.dma_start(out=wo1_sb[:], in_=wo1_t[:])
        betaf_sb = persist.tile([P, 2], F32)
        nc.sync.dma_start(out=betaf_sb[:], in_=beta_f[:])
        betao_sb = persist.tile([P, 2], F32)
        nc.sync.dma_start(out=betao_sb[:], in_=beta_o[:])
        cnt_sb = persist.tile([P, 1], F32)
        nc.sync.dma_start(out=cnt_sb[:], in_=cnt_er[:])

        # big feature load on the SWDGE queue, blk-interleaved so pass-1
        # h-chunks unblock as early as possible
        feat = persist.tile([P, 2 * HW], F32R)   # c-blk0 | c-blk1, cast fp32->fp32r
        for q in range(8):
            for blk in range(2):
                nc.gpsimd.dma_start(
                    out=feat[:, blk * HW + q * 2048: blk * HW + (q + 1) * 2048],
                    in_=feature[blk * P:(blk + 1) * P, q * 2048:(q + 1) * 2048])

        # persistent results of phase 1
        fbuT = persist.tile([P, P * 12], F32R)     # [w, h*12 + k]
        wc_sb = persist.tile([P, 512], F32R)       # Wc^T blocks (a*2+o)
        u_sb = persist.tile([P, 2], F32)           # bias per o-blk

        # ---------------- morphology ----------------
        with tc.tile_pool(name="morph", bufs=1) as mo, \
             tc.tile_pool(name="morph_ps", bufs=2, space="PSUM") as mops:
            z_sb = mo.tile([P, N * P], F32R)
            nc.vector.tensor_scalar(z_sb[:], m_sb[:], 0.3, None, op0=ALU.is_gt)

            ps_rows_er = mops.tile([P, N * P], F32, tag="mps")
            nc.tensor.matmul(ps_rows_er[:], bander_sb, z_sb[:],
                             start=True, stop=True)
            ps_rows_dl = mops.tile([P, N * P], F32, tag="mps")
            nc.tensor.matmul(ps_rows_dl[:], banddl_sb, z_sb[:],
                             start=True, stop=True)

            rows_er = mo.tile([P, N * P], F32R)
            nc.vector.tensor_scalar(rows_er[:], ps_rows_er[:], cnt_sb[:, 0:1], None,
                                    op0=ALU.is_equal)
            rows_dl = mo.tile([P, N * P], F32R)
            nc.vector.tensor_scalar(rows_dl[:], ps_rows_dl[:], 0.5, None,
                                    op0=ALU.is_gt)

            # transpose each class tile -> [w, h]
            rows_erT = mo.tile([P, N * P], F32R)
            rows_dlT = mo.tile([P, N * P], F32R)
            for n in range(N):
                ps_tr = mops.tile([P, 2 * P], F32R, tag="mps")
                nc.tensor.matmul(ps_tr[:, 0:P], rows_er[:, n * P:(n + 1) * P],
                                 ident_r, is_transpose=True)
                nc.tensor.matmul(ps_tr[:, P:2 * P], rows_dl[:, n * P:(n + 1) * P],
                                 ident_r, is_transpose=True)
                nc.vector.tensor_copy(rows_erT[:, n * P:(n + 1) * P], ps_tr[:, 0:P])
                nc.vector.tensor_copy(rows_dlT[:, n * P:(n + 1) * P], ps_tr[:, P:2 * P])

            ps_cols_er = mops.tile([P, N * P], F32, tag="mps")
            nc.tensor.matmul(ps_cols_er[:], bander_sb, rows_erT[:],
                             start=True, stop=True)
            ps_cols_dl = mops.tile([P, N * P], F32, tag="mps")
            nc.tensor.matmul(ps_cols_dl[:], banddl_sb, rows_dlT[:],
                             start=True, stop=True)

            er_t = mo.tile([P, N * P], F32)   # er^T per class [w, h]
            dl_t = mo.tile([P, N * P], F32)
            nc.vector.tensor_scalar(er_t[:], ps_cols_er[:], cnt_sb[:, 0:1], None,
                                    op0=ALU.is_equal)
            nc.vector.tensor_scalar(dl_t[:], ps_cols_dl[:], 0.5, None, op0=ALU.is_gt)

            # write channels into fbuT at [w, h*12 + k], k = 3n+j
            fbuT_v = fbuT.rearrange("w (h k) -> w h k", k=12)
            for n in range(N):
                src_er = er_t[:, n * P:(n + 1) * P]
                src_dl = dl_t[:, n * P:(n + 1) * P]
                nc.vector.tensor_copy(fbuT_v[:, :, 3 * n], src_er)
                nc.vector.tensor_scalar(fbuT_v[:, :, 3 * n + 1], src_dl, 0.0, None,
                                        op0=ALU.is_equal)
                nc.vector.tensor_tensor(fbuT_v[:, :, 3 * n + 2], src_dl, src_er,
                                        op=ALU.subtract)

        # ---------------- pass 1: feature transpose + mid ----------------
        mid_r = persist.tile([12, 256], F32R)
        with tc.tile_pool(name="mid_ps", bufs=1, space="PSUM") as midps, \
             tc.tile_pool(name="p1_ps", bufs=2, space="PSUM") as p1ps, \
             tc.tile_pool(name="p1_sb", bufs=3) as p1sb:
            mid_ps = midps.tile([12, 256], F32)
            for g in range(P // G1):
                tr = p1ps.tile([P, G1 * 256], F32R, tag="tr")
                for j in range(G1):
                    h = g * G1 + j
                    nc.tensor.matmul(tr[:, j * 256:j * 256 + P],
                                     feat[:, h * P:(h + 1) * P],
                                     ident_r, is_transpose=True)
                    nc.tensor.matmul(tr[:, j * 256 + P:(j + 1) * 256],
                                     feat[:, HW + h * P:HW + (h + 1) * P],
                                     ident_r, is_transpose=True)
                ft = p1sb.tile([P, G1 * 256], F32R, tag="ft")
                if g % 2 == 0:
                    nc.vector.tensor_copy(ft[:], tr[:])
                else:
                    nc.scalar.copy(ft[:], tr[:])
                for j in range(G1):
                    h = g * G1 + j
                    nc.tensor.matmul(mid_ps[:], fbuT[:, h * 12:h * 12 + 12],
                                     ft[:, j * 256:(j + 1) * 256],
                                     start=(h == 0), stop=(h == P - 1),
                                     skip_group_check=True)
            nc.vector.tensor_copy(mid_r[:], mid_ps[:])

        # ---------------- small stage: mid^T, A'^T, G^T, Wc, u ----------------
        with tc.tile_pool(name="sm_ps", bufs=1, space="PSUM") as smps, \
             tc.tile_pool(name="sm_sb", bufs=1) as smsb:
            # mid^T via PE transpose of [12,128] chunks (fp32r)
            ps_mt = smps.tile([P, 24], F32R, tag="mt")
            for ci in range(2):
                nc.tensor.matmul(ps_mt[:, ci * 12:(ci + 1) * 12],
                                 mid_r[:, ci * P:(ci + 1) * P],
                                 pk_sb[0:12, 1280:1292], is_transpose=True)
            mid_t = smsb.tile([P, 24], F32R)
            nc.vector.tensor_copy(mid_t[:], ps_mt[:])

            # A'^T = mid @ Wo2'^T   [12, 256]
            ps_at = smps.tile([12, 256], F32, tag="at")
            nc.tensor.matmul(ps_at[:], mid_t[:, 0:12], wo2_sb[:, 0:256],
                             start=True, stop=False)
            nc.tensor.matmul(ps_at[:], mid_t[:, 12:24], wo2_sb[:, 256:512],
                             start=False, stop=True)
            a_t = smsb.tile([12, 256], F32R)
            nc.vector.tensor_copy(a_t[:], ps_at[:])

            # G^T[c, o] = sum_k mid[k, c] A'^T[k, o];  chunks ci on partitions
            ps_gt = smps.tile([P, 512], F32, tag="gt")
            for ci in range(2):
                nc.tensor.matmul(ps_gt[:, ci * 256:(ci + 1) * 256],
                                 mid_r[:, ci * P:(ci + 1) * P], a_t[:],
                                 start=True, stop=True)
            gt_r = smsb.tile([P, 512], F32R)
            nc.vector.tensor_copy(gt_r[:], ps_gt[:])
            gt_f = smsb.tile([P, 512], F32)
            nc.vector.tensor_copy(gt_f[:], ps_gt[:])

            # X = Wf'^T @ G^T (= (G Wf')^T); blocks a (c_in chunk) on partitions
            ps_x = smps.tile([P, 512], F32, tag="x")
            for a in range(2):
                for ci in range(2):
                    nc.tensor.matmul(ps_x[:, a * 256:(a + 1) * 256],
                                     wfn_sb[:, (ci * 2 + a) * P:(ci * 2 + a + 1) * P],
                                     gt_r[:, ci * 256:(ci + 1) * 256],
                                     start=(ci == 0), stop=(ci == 1),
                                     skip_group_check=True)
            # Wc^T = Wo1'^T + X  (blocks (a*2+o) align with [a*256 + o*128])
            for a in range(2):
                nc.vector.tensor_tensor(wc_sb[:, a * 256:(a + 1) * 256],
                                        ps_x[:, a * 256:(a + 1) * 256],
                                        wo1_sb[:, a * 256:(a + 1) * 256],
                                        op=ALU.add)

            # u = G @ beta_f + beta_o   per o-blk  (fp32 matmuls)
            for o in range(2):
                ps_u = smps.tile([P, 1], F32, tag="u")
                nc.tensor.matmul(ps_u[:], gt_f[:, o * P:(o + 1) * P],
                                 betaf_sb[:, 0:1], start=True, stop=False)
                nc.tensor.matmul(ps_u[:], gt_f[:, 256 + o * P:256 + (o + 1) * P],
                                 betaf_sb[:, 1:2], start=False, stop=True)
                nc.scalar.activation(u_sb[:, o:o + 1], ps_u[:], ACTF.Identity,
                                     bias=betao_sb[:, o:o + 1])

        # ---------------- pass 2: out = Wc @ f + u ----------------
        with tc.tile_pool(name="out_ps", bufs=4, space="PSUM") as outps, \
             tc.tile_pool(name="p2_sb", bufs=2) as p2sb:
            for tg in range(NPT // 4):
                ot0 = p2sb.tile([P, 4 * PT], F32, tag="ot0")
                ot1 = p2sb.tile([P, 4 * PT], F32, tag="ot1")
                for tt in range(4):
                    t = tg * 4 + tt
                    c0 = t * PT
                    out_ps = outps.tile([P, 2 * PT], F32, tag="ops")
                    for o in range(2):
                        ops = out_ps[:, o * PT:(o + 1) * PT]
                        nc.tensor.matmul(ops,
                                         wc_sb[:, (0 * 2 + o) * P:(0 * 2 + o + 1) * P],
                                         feat[:, c0:c0 + PT],
                                         start=True, stop=False, skip_group_check=True)
                        nc.tensor.matmul(ops,
                                         wc_sb[:, (1 * 2 + o) * P:(1 * 2 + o + 1) * P],
                                         feat[:, HW + c0:HW + c0 + PT],
                                         start=False, stop=True, skip_group_check=True)
                    nc.scalar.activation(ot0[:, tt * PT:(tt + 1) * PT],
                                         out_ps[:, 0:PT],
                                         ACTF.Identity, bias=u_sb[:, 0:1])
                    nc.vector.tensor_scalar(ot1[:, tt * PT:(tt + 1) * PT],
                                            out_ps[:, PT:2 * PT],
                                            u_sb[:, 1:2], None, op0=ALU.add)
                g0 = tg * 4 * PT
                nc.sync.dma_start(out=out[0:P, g0:g0 + 4 * PT], in_=ot0[:])
                nc.scalar.dma_start(out=out[P:C, g0:g0 + 4 * PT], in_=ot1[:])

    nc.compile()
    _NC_CACHE["nc"] = nc
    return nc


def prepare_in_maps(feature, m, W_f, g_f, b_f, mu_f, v_f, W_o, g_o, b_o, mu_o, v_o):
    feature = np.asarray(feature, dtype=np.float32)
    m = np.asarray(m, dtype=np.float32)
    W_f = np.asarray(W_f, dtype=np.float32)
    W_o = np.asarray(W_o, dtype=np.float32)
    g_f, b_f, mu_f, v_f = (np.asarray(x, dtype=np.float32) for x in (g_f, b_f, mu_f, v_f))
    g_o, b_o, mu_o, v_o = (np.asarray(x, dtype=np.float32) for x in (g_o, b_o, mu_o, v_o))

    inv_f = g_f / np.sqrt(v_f + EPS)
    beta_f_v = b_f - mu_f * inv_f
    inv_o = g_o / np.sqrt(v_o + EPS)
    beta_o_v = b_o - mu_o * inv_o
    Wf_p = (inv_f[:, None] * W_f).astype(np.float32)          # [C, C]
    Wo1_p = (inv_o[:, None] * W_o[:, :C]).astype(np.float32)  # [C, C]
    Wo2_p = (inv_o[:, None] * W_o[:, C:]).astype(np.float32)  # [C, C]

    def blocks_t(Wp):
        # lhsT layout: blocks ci*2+o of Wp^T
        a = np.empty((P, 512), np.float32)
        for ci in range(2):
            for o in range(2):
                a[:, (ci * 2 + o) * P:(ci * 2 + o + 1) * P] = \
                    Wp[o * P:(o + 1) * P, ci * P:(ci + 1) * P].T
        return a

    def blocks_n(Wp):
        # natural-layout blocks ci*2+a: Wp[ci*128:(ci+1)*128, a*128:(a+1)*128]
        a_ = np.empty((P, 512), np.float32)
        for ci in range(2):
            for a in range(2):
                a_[:, (ci * 2 + a) * P:(ci * 2 + a + 1) * P] = \
                    Wp[ci * P:(ci + 1) * P, a * P:(a + 1) * P]
        return a_

    band_er, band_dl, cnt_er = _band_consts()
    pk = np.empty((P, PK_W), np.float32)
    pk[:, 0:512] = np.concatenate([Wo2_p.T[0:P, :], Wo2_p.T[P:C, :]], axis=1)
    pk[:, 512:1024] = blocks_n(Wf_p)
    pk[:, 1024:1152] = band_er
    pk[:, 1152:1280] = band_dl
    pk[:, 1280:1408] = np.eye(P, dtype=np.float32)

    common = {
        "pk": pk,
        "wo1_t": blocks_t(Wo1_p),
        "beta_f": beta_f_v.reshape(2, P).T.copy(),
        "beta_o": beta_o_v.reshape(2, P).T.copy(),
        "cnt_er": cnt_er,
    }
    in_maps = []
    for b in range(B):
        im = dict(common)
        im["feature"] = np.ascontiguousarray(feature[b].reshape(C, HW))
        im["m"] = np.ascontiguousarray(m[b])
        in_maps.append(im)
    return in_maps


def kernel(feature, m, W_f, g_f, b_f, mu_f, v_f, W_o, g_o, b_o, mu_o, v_o):
    nc = build()
    in_maps = prepare_in_maps(feature, m, W_f, g_f, b_f, mu_f, v_f,
                              W_o, g_o, b_o, mu_o, v_o)
    res = bass_utils.run_bass_kernel_spmd(nc, in_maps, list(range(B)))
    out = np.empty((B, C, H, W), np.float32)
    for b in range(B):
        out[b] = res.results[b]["out"].reshape(C, H, W)
    return out
